# revision 2
# baseline (speedup 1.0000x reference)
"""MoE feed-forward (8 experts, top-2) on 8 TRN2 NeuronCores, expert-parallel.

Strategy: core c holds expert c's weights. Tokens are sharded by position
(1024/core). Each core computes fp32 gating + top-2 for its tokens, assigns
per-expert slots via triangular-matmul prefix sums, scatters bf16 token rows
into an [E, C, D] send buffer with indirect DMA, AllToAll-dispatches them,
runs the expert MLP in bf16 (fp32 accumulate), AllToAll-returns fp32 results,
then gathers its tokens' two expert outputs and combines with the renormalized
routing weights.
"""
import numpy as np

import concourse.bass as bass
import concourse.mybir as mybir
import concourse.tile as tile
from concourse import bacc
from concourse.bass import IndirectOffsetOnAxis
from concourse.bass_utils import run_bass_kernel_spmd
from concourse.masks import make_identity, make_upper_triangular

D_MODEL, HIDDEN, N_EXPERTS, TOP_K = 1024, 4096, 8, 2
N_CORES = 8
P = 128
T = 8192
T_LOC = T // N_CORES            # 1024 tokens per core
N_TOK_TILES = T_LOC // P        # 8
D_BLKS = D_MODEL // P           # 8
H_BLKS = HIDDEN // P            # 32
N_CT = 256                      # token tile in expert-compute phase

FP32 = mybir.dt.float32
BF16 = mybir.dt.bfloat16
I32 = mybir.dt.int32
U32 = mybir.dt.uint32
AF = mybir.ActivationFunctionType
ALU = mybir.AluOpType

RG = [list(range(N_CORES))]


def _body(tc, C, x_loc, gate_w, gate_b_rep, iota8_rep, w1_loc, b1_loc, w2_loc,
          b2_rep, out_loc):
    nc = tc.nc
    S = N_EXPERTS * C

    with tc.tile_pool(name="dram", bufs=1, space="DRAM") as dram, \
         tc.tile_pool(name="persist", bufs=1) as persist:
        send_x = dram.tile([S, D_MODEL], BF16)
        recv_x = dram.tile([S, D_MODEL], BF16)
        send_y = dram.tile([S, D_MODEL], FP32)
        recv_y = dram.tile([S, D_MODEL], FP32)

        ident = persist.tile([P, P], FP32)
        make_identity(nc, ident)
        strictu = persist.tile([P, P], FP32)
        make_upper_triangular(nc, strictu, val=1.0, diag=False)
        ones_t = persist.tile([P, P], FP32)
        nc.gpsimd.memset(ones_t, 1.0)

        gb_sb = persist.tile([P, N_EXPERTS], FP32)
        nc.sync.dma_start(gb_sb, gate_b_rep[:])
        iota_sb = persist.tile([P, N_EXPERTS], FP32)
        nc.sync.dma_start(iota_sb, iota8_rep[:])
        gw_sb = persist.tile([P, D_BLKS, N_EXPERTS], FP32)
        nc.sync.dma_start(gw_sb, gate_w[:].rearrange("(j p) e -> p j e", p=P))
        b1_sb = persist.tile([P, H_BLKS], FP32)
        nc.sync.dma_start(b1_sb, b1_loc[:])
        b2r_sb = persist.tile([P, D_MODEL], FP32)
        nc.sync.dma_start(b2r_sb, b2_rep[:])

        rows_sb = persist.tile([P, N_TOK_TILES, TOP_K], I32)
        wts_sb = persist.tile([P, N_TOK_TILES, TOP_K], FP32)
        sendmask = persist.tile([P, N_TOK_TILES, N_EXPERTS], FP32)
        x_bf_all = persist.tile([P, N_TOK_TILES, D_MODEL], BF16)

        w1_sb = persist.tile([P, D_BLKS, HIDDEN], BF16)
        w2_sb = persist.tile([P, H_BLKS, D_MODEL], BF16)

        # ---- expert weights: load fp32 (scalar HWDGE queue, so phase-A loads
        # on the sync queue aren't stalled behind 32MB), cast to bf16 ----
        W_CHUNK = 2048
        with tc.tile_pool(name="wstage", bufs=3) as wstage:
            for j in range(D_BLKS):
                for h in range(HIDDEN // W_CHUNK):
                    wst = wstage.tile([P, W_CHUNK], FP32, tag="wst", name="wst")
                    nc.scalar.dma_start(wst, w1_loc[j * P:(j + 1) * P,
                                                    h * W_CHUNK:(h + 1) * W_CHUNK])
                    nc.vector.tensor_copy(
                        w1_sb[:, j, h * W_CHUNK:(h + 1) * W_CHUNK], wst)
            for m in range(H_BLKS):
                wst = wstage.tile([P, W_CHUNK], FP32, tag="wst", name="wst")
                nc.scalar.dma_start(wst[:, :D_MODEL], w2_loc[m * P:(m + 1) * P, :])
                nc.vector.tensor_copy(w2_sb[:, m, :], wst[:, :D_MODEL])

            # ---- phase A: gating + routing + dispatch scatter ----
            with tc.tile_pool(name="phA", bufs=2) as pA, \
                 tc.tile_pool(name="phA_psum", bufs=2, space="PSUM") as pAp:
                for i in range(N_TOK_TILES):
                    x_sb = pA.tile([P, D_MODEL], FP32, tag="x_sb", name="x_sb")
                    nc.sync.dma_start(x_sb, x_loc[i * P:(i + 1) * P, :])
                    nc.vector.tensor_copy(x_bf_all[:, i, :], x_sb)

                    xT = pA.tile([P, D_BLKS, P], FP32, tag="xT", name="xT")
                    for j in range(D_BLKS):
                        tp = pAp.tile([P, P], FP32, tag="tp", name="tp")
                        nc.tensor.transpose(tp, x_sb[:, j * P:(j + 1) * P], ident)
                        nc.vector.tensor_copy(xT[:, j, :], tp)

                    gps = pAp.tile([P, N_EXPERTS], FP32, tag="gps", name="gps")
                    for j in range(D_BLKS):
                        nc.tensor.matmul(gps, lhsT=xT[:, j, :], rhs=gw_sb[:, j, :],
                                         start=(j == 0), stop=(j == D_BLKS - 1))
                    logits = pA.tile([P, N_EXPERTS], FP32, tag="logits", name="logits")
                    nc.vector.tensor_add(logits, gps, gb_sb)

                    maxv = pA.tile([P, 8], FP32, tag="maxv", name="maxv")
                    nc.vector.max(maxv, logits)
                    maxi = pA.tile([P, 8], U32, tag="maxi", name="maxi")
                    nc.vector.max_index(maxi, maxv, logits)

                    d01 = pA.tile([P, 1], FP32, tag="d01", name="d01")
                    nc.vector.tensor_sub(d01, maxv[:, 0:1], maxv[:, 1:2])
                    # renormalized top-2: w0 = sigmoid(l0-l1), w1 = sigmoid(l1-l0)
                    nc.scalar.activation(wts_sb[:, i, 0:1], d01, AF.Sigmoid)
                    nc.scalar.activation(wts_sb[:, i, 1:2], d01, AF.Sigmoid,
                                         scale=-1.0)

                    idxf = pA.tile([P, TOP_K], FP32, tag="idxf", name="idxf")
                    nc.vector.tensor_copy(idxf, maxi[:, 0:TOP_K])
                    eq0 = pA.tile([P, N_EXPERTS], FP32, tag="eq0", name="eq0")
                    nc.vector.tensor_tensor(
                        eq0, idxf[:, 0:1].to_broadcast([P, N_EXPERTS]),
                        iota_sb, op=ALU.is_equal)
                    eq1 = pA.tile([P, N_EXPERTS], FP32, tag="eq1", name="eq1")
                    nc.vector.tensor_tensor(
                        eq1, idxf[:, 1:2].to_broadcast([P, N_EXPERTS]),
                        iota_sb, op=ALU.is_equal)
                    nc.vector.tensor_add(sendmask[:, i, :], eq0, eq1)

                    # exclusive prefix count of earlier tokens per expert:
                    # pref = sum_{j<i} colsum(mask_j) + strict_prefix(mask_i)
                    pref = pAp.tile([P, N_EXPERTS], FP32, tag="pref", name="pref")
                    for j in range(i + 1):
                        nc.tensor.matmul(pref,
                                         lhsT=(strictu if j == i else ones_t),
                                         rhs=sendmask[:, j, :],
                                         start=(j == 0), stop=(j == i))
                    offs = pA.tile([P, N_EXPERTS], FP32, tag="offs", name="offs")
                    nc.vector.tensor_copy(offs, pref)

                    for k in range(TOP_K):
                        eqk = eq0 if k == 0 else eq1
                        prod = pA.tile([P, N_EXPERTS], FP32, tag="prod", name="prod")
                        nc.vector.tensor_mul(prod, offs, eqk)
                        slot = pA.tile([P, 1], FP32, tag="slot", name="slot")
                        nc.vector.reduce_sum(slot, prod, axis=mybir.AxisListType.X)
                        rowf = pA.tile([P, 1], FP32, tag="rowf", name="rowf")
                        nc.vector.tensor_scalar(rowf, idxf[:, k:k + 1], float(C),
                                                slot, op0=ALU.mult, op1=ALU.add)
                        nc.vector.tensor_copy(rows_sb[:, i, k:k + 1], rowf)
                        nc.gpsimd.indirect_dma_start(
                            out=send_x[:],
                            out_offset=IndirectOffsetOnAxis(
                                ap=rows_sb[:, i, k:k + 1], axis=0),
                            in_=x_bf_all[:, i, :],
                            in_offset=None,
                        )

        # ---- dispatch all-to-all ----
        nc.gpsimd.collective_compute(
            "AllToAll", ALU.bypass, replica_groups=RG,
            ins=[send_x[:].opt()], outs=[recv_x[:].opt()])

        # ---- phase C: expert MLP over S = E*C slots ----
        with tc.tile_pool(name="phC", bufs=2) as pC, \
             tc.tile_pool(name="phC_psum", bufs=2, space="PSUM") as pCp:
            for n in range(S // N_CT):
                xrT = pC.tile([P, D_BLKS, N_CT], BF16, tag="xrT", name="xrT")
                for j in range(D_BLKS):
                    nc.sync.dma_start(
                        xrT[:, j, :],
                        recv_x[n * N_CT:(n + 1) * N_CT, j * P:(j + 1) * P],
                        transpose=True)
                hT = pC.tile([P, H_BLKS, N_CT], BF16, tag="hT", name="hT")
                for m in range(H_BLKS):
                    ps1 = pCp.tile([P, N_CT], FP32, tag="ps1", name="ps1")
                    for j in range(D_BLKS):
                        nc.tensor.matmul(ps1,
                                         lhsT=w1_sb[:, j, m * P:(m + 1) * P],
                                         rhs=xrT[:, j, :],
                                         start=(j == 0), stop=(j == D_BLKS - 1))
                    nc.scalar.activation(hT[:, m, :], ps1, AF.Silu,
                                         bias=b1_sb[:, m:m + 1])
                for t in range(N_CT // P):
                    y_tm = pC.tile([P, D_MODEL], FP32, tag="y_tm", name="y_tm")
                    for nh in range(2):
                        ps2 = pCp.tile([P, 512], FP32, tag="ps2", name="ps2")
                        for m in range(H_BLKS):
                            nc.tensor.matmul(
                                ps2, lhsT=hT[:, m, t * P:(t + 1) * P],
                                rhs=w2_sb[:, m, nh * 512:(nh + 1) * 512],
                                start=(m == 0), stop=(m == H_BLKS - 1))
                        nc.vector.tensor_add(y_tm[:, nh * 512:(nh + 1) * 512],
                                             ps2, b2r_sb[:, nh * 512:(nh + 1) * 512])
                    r0 = n * N_CT + t * P
                    nc.sync.dma_start(send_y[r0:r0 + P, :], y_tm)

        # ---- return all-to-all ----
        nc.gpsimd.collective_compute(
            "AllToAll", ALU.bypass, replica_groups=RG,
            ins=[send_y[:].opt()], outs=[recv_y[:].opt()])

        # ---- phase E: gather + weighted combine ----
        with tc.tile_pool(name="phE", bufs=2) as pE:
            for i in range(N_TOK_TILES):
                g0 = pE.tile([P, D_MODEL], FP32, tag="g0", name="g0")
                nc.gpsimd.indirect_dma_start(
                    out=g0, out_offset=None, in_=recv_y[:],
                    in_offset=IndirectOffsetOnAxis(ap=rows_sb[:, i, 0:1], axis=0))
                g1 = pE.tile([P, D_MODEL], FP32, tag="g1", name="g1")
                nc.gpsimd.indirect_dma_start(
                    out=g1, out_offset=None, in_=recv_y[:],
                    in_offset=IndirectOffsetOnAxis(ap=rows_sb[:, i, 1:2], axis=0))
                t0 = pE.tile([P, D_MODEL], FP32, tag="t0", name="t0")
                nc.vector.tensor_scalar_mul(t0, g0, wts_sb[:, i, 0:1])
                t1 = pE.tile([P, D_MODEL], FP32, tag="t1", name="t1")
                nc.vector.tensor_scalar_mul(t1, g1, wts_sb[:, i, 1:2])
                out_t = pE.tile([P, D_MODEL], FP32, tag="out_t", name="out_t")
                nc.vector.tensor_add(out_t, t0, t1)
                nc.sync.dma_start(out_loc[i * P:(i + 1) * P, :], out_t)


def build_kernel(C):
    nc = bacc.Bacc("TRN2", target_bir_lowering=False, debug=False,
                   num_devices=N_CORES)
    args = dict(
        x_loc=nc.dram_tensor("x_loc", [T_LOC, D_MODEL], FP32, kind="ExternalInput"),
        gate_w=nc.dram_tensor("gate_w", [D_MODEL, N_EXPERTS], FP32, kind="ExternalInput"),
        gate_b_rep=nc.dram_tensor("gate_b_rep", [P, N_EXPERTS], FP32, kind="ExternalInput"),
        iota8_rep=nc.dram_tensor("iota8_rep", [P, N_EXPERTS], FP32, kind="ExternalInput"),
        w1_loc=nc.dram_tensor("w1_loc", [D_MODEL, HIDDEN], FP32, kind="ExternalInput"),
        b1_loc=nc.dram_tensor("b1_loc", [P, H_BLKS], FP32, kind="ExternalInput"),
        w2_loc=nc.dram_tensor("w2_loc", [HIDDEN, D_MODEL], FP32, kind="ExternalInput"),
        b2_rep=nc.dram_tensor("b2_rep", [P, D_MODEL], FP32, kind="ExternalInput"),
        out_loc=nc.dram_tensor("out_loc", [T_LOC, D_MODEL], FP32, kind="ExternalOutput"),
    )
    with tile.TileContext(nc) as tc:
        _body(tc, C, **{k: v.ap() for k, v in args.items()})
    nc.compile()
    return nc


def _capacity(flat_x, gate_w, gate_b):
    """Max tokens any (src core, expert) pair routes, from the actual input."""
    logits = flat_x @ gate_w + gate_b
    top2 = np.argsort(-logits, axis=1, kind="stable")[:, :TOP_K]
    blocks = top2.reshape(N_CORES, T_LOC, TOP_K)
    counts = np.stack([(blocks == e).sum(axis=(1, 2)) for e in range(N_EXPERTS)])
    max_cnt = int(counts.max())
    # +8 margin vs tiny fp reorder flips between host and device gating,
    # rounded to 32 so S = 8C divides the 256-token compute tile.
    return ((max_cnt + 8 + 31) // 32) * 32


_CACHE = {}


def kernel(x, gate_w, gate_b, w1, b1, w2, b2, _trace=False):
    x = np.ascontiguousarray(np.asarray(x, dtype=np.float32))
    gate_w = np.ascontiguousarray(np.asarray(gate_w, dtype=np.float32))
    gate_b = np.ascontiguousarray(np.asarray(gate_b, dtype=np.float32))
    w1 = np.ascontiguousarray(np.asarray(w1, dtype=np.float32))
    b1 = np.ascontiguousarray(np.asarray(b1, dtype=np.float32))
    w2 = np.ascontiguousarray(np.asarray(w2, dtype=np.float32))
    b2 = np.ascontiguousarray(np.asarray(b2, dtype=np.float32))

    orig_shape = x.shape
    flat_x = x.reshape(-1, D_MODEL)
    C = _capacity(flat_x, gate_w, gate_b)

    if C not in _CACHE:
        _CACHE[C] = build_kernel(C)
    nc = _CACHE[C]

    iota8 = np.tile(np.arange(N_EXPERTS, dtype=np.float32), (P, 1))
    gb_rep = np.tile(gate_b, (P, 1))
    in_maps = []
    for c in range(N_CORES):
        in_maps.append({
            "x_loc": flat_x[c * T_LOC:(c + 1) * T_LOC],
            "gate_w": gate_w,
            "gate_b_rep": gb_rep,
            "iota8_rep": iota8,
            "w1_loc": w1[c],
            "b1_loc": np.ascontiguousarray(b1[c].reshape(H_BLKS, P).T),
            "w2_loc": w2[c],
            "b2_rep": np.tile(b2[c], (P, 1)),
        })

    res = run_bass_kernel_spmd(nc, in_maps, core_ids=list(range(N_CORES)),
                               trace=_trace)
    out = np.concatenate([res.results[c]["out_loc"] for c in range(N_CORES)],
                         axis=0)
    if _trace:
        kernel.last_results = res
    return out.reshape(orig_shape)
